# revision 15
# baseline (speedup 1.0000x reference)
"""GRU decoder kernel for Trainium2 (8 NeuronCores, data-parallel over batch).

v1 design (vs v0 baseline):
 - bf16 weights (moving operands; same 1 cyc/row as f32r, half the DMA/SBUF),
   bf16 hidden state + EW intermediates (4x-rate DVE ops on SBUF operands),
   bf16 transposes (1 cyc/row vs 2 for f32).
 - Per-wave program order keeps single-buffered hT tiles correct: every
   consumer of h_i(t-1) reads before the EW that overwrites it.
 - Cells 1+2 share one [128, 2048] psum arena (partition halves) so their
   EW chain runs fused at [128, *] width - half the instruction count.
 - Cell 0 has its own [64, 1536] psum (rz 0:1002, n 1024:1526); gi0 is
   accumulated by a cheap identity matmul (f32r), gi0 n-part adds from SBUF.
 - h2 history lives in SBUF (121 slots, bf16): cell2's gh reads slot t-1
   directly, and the fc2+softmax epilogue runs post-scan from SBUF in
   8-timestep batches - no DRAM roundtrip.
"""

import os
import sys

sys.path.insert(0, "/opt/trn_rl_repo")

import numpy as np

import concourse.bass as bass
import concourse.mybir as mybir
import concourse.tile as tile
from concourse import bacc
from concourse import bass_utils
from concourse.masks import make_identity

F32 = mybir.dt.float32
F32R = mybir.dt.float32r
BF16 = mybir.dt.bfloat16
AX = mybir.AxisListType
ALU = mybir.AluOpType
ACTF = mybir.ActivationFunctionType

D_LATENT = 292
D_CHAR = 35
H = 501
G = 3 * H  # 1503
GP = 1504
T = int(os.environ.get("BASS_GRU_T", "120"))
BATCH = 512
NCORES = 8
BC = BATCH // NCORES  # 64

SELU_L = 1.0507009873554804934193349852946
SELU_A = 1.6732632423543772848170429916717

# rz gate columns (psum 0:1002), chunks bank-aligned
RZCH = [(0, 512), (512, 490)]
N_SRC = 1002          # n-gate columns in the weight array
N_W = 502             # padded n width
N_GH = 1024           # n(gh) psum offset (bank 2)
N_GI = 1536           # n(gi) psum offset (bank 3)
KC = 4
ONES_ROW = 501        # chunk 3, row 117
KCX = 3
ONES_ROW_X = 292      # chunk 2, row 36

EPI_TB = 8            # fc2/softmax timesteps per epilogue chunk

_CACHE = {}


def build_bass():
    nc = bacc.Bacc("TRN2", target_bir_lowering=False, debug=False)

    # ---- DRAM I/O ----
    z_in = nc.dram_tensor("z_in", [BC, D_LATENT], F32, kind="ExternalInput").ap()
    w1s_d = nc.dram_tensor("w1s", [128, KCX, D_LATENT], F32R, kind="ExternalInput").ap()
    wih0_d = nc.dram_tensor("wih0s", [128, KCX, GP], BF16, kind="ExternalInput").ap()
    whh0_d = nc.dram_tensor("whh0s", [128, KC, GP], BF16, kind="ExternalInput").ap()
    wih1_d = nc.dram_tensor("wih1s", [128, KC, GP], BF16, kind="ExternalInput").ap()
    whh1_d = nc.dram_tensor("whh1s", [128, KC, GP], BF16, kind="ExternalInput").ap()
    wih2_d = nc.dram_tensor("wih2s", [128, KC, GP], BF16, kind="ExternalInput").ap()
    whh2_d = nc.dram_tensor("whh2s", [128, KC, GP], BF16, kind="ExternalInput").ap()
    w2s_d = nc.dram_tensor("w2s", [128, KC, 36], BF16, kind="ExternalInput").ap()
    TS = T + 1  # hist2 slots (slot 0 = h2(-1) = zeros)
    onesb_d = nc.dram_tensor("onesb", [1, TS * 64], BF16, kind="ExternalInput").ap()
    onesr_d = nc.dram_tensor("onesr", [1, 64], F32R, kind="ExternalInput").ap()
    probs = nc.dram_tensor("probs", [BC, T, D_CHAR], F32, kind="ExternalOutput").ap()

    with tile.TileContext(nc) as tc:
        with tc.tile_pool(name="singles", bufs=1) as sg:
            # ---- weights (order matters: prologue needs w1s+wih0 first) ----
            w1s = sg.tile([128, KCX, D_LATENT], F32R)
            wih0 = sg.tile([128, KCX, GP], BF16)
            whh0 = sg.tile([128, KC, GP], BF16)
            whh1 = sg.tile([128, KC, GP], BF16)
            wih1 = sg.tile([128, KC, GP], BF16)
            whh2 = sg.tile([128, KC, GP], BF16)
            wih2 = sg.tile([128, KC, GP], BF16)
            w2s = sg.tile([128, KC, 36], BF16)
            for dst, src in [
                (w1s, w1s_d), (wih0, wih0_d), (whh0, whh0_d), (whh1, whh1_d),
                (wih1, wih1_d), (whh2, whh2_d), (wih2, wih2_d), (w2s, w2s_d),
            ]:
                nc.sync.dma_start(out=dst, in_=src)

            ident = sg.tile([128, 128], F32)
            make_identity(nc, ident)
            id64 = ident[0:64, 0:64]
            identb = sg.tile([128, 128], BF16)
            nc.scalar.copy(out=identb, in_=ident)

            # ---- persistent state (bf16) ----
            h0T = sg.tile([128, KC, BC], BF16)   # h0(t-1), hidden-major
            h1T = sg.tile([128, KC, BC], BF16)
            hist2 = sg.tile([128, TS, KC, BC], BF16)  # h2 history (slot t+1 = h2(t))
            h12b = sg.tile([128, H], BF16)       # prev h, batch-major (c1|c2)
            h0b = sg.tile([BC, H], BF16)
            gi0rz = sg.tile([BC, 1002], BF16)
            gi0n = sg.tile([BC, N_W], BF16)

            for t_ in (h0T, h1T):
                nc.vector.memset(t_, 0.0)
                nc.sync.dma_start(out=t_[117:118, 3, :],
                                  in_=onesb_d[:, 0:64])  # ones row (idx 501)
            nc.vector.memset(hist2, 0.0)
            nc.sync.dma_start(
                out=hist2[117:118, :, 3, :],
                in_=onesb_d.rearrange("o (t b) -> o t b", t=TS))
            nc.vector.memset(h12b, 0.0)
            nc.vector.memset(h0b, 0.0)

            # ================= prologue: x = selu(fc1(z)); gi0 = x @ wih0 ====
            with tc.tile_pool(name="ppsum", bufs=1, space="PSUM") as pp, \
                 tc.tile_pool(name="ptmp", bufs=1) as pt:
                zsb = pt.tile([BC, D_LATENT], F32)
                nc.sync.dma_start(out=zsb, in_=z_in)
                trp = pp.tile([128, KCX, BC], F32)
                zT = pt.tile([128, KCX, BC], F32R)
                uT = pt.tile([128, KCX, BC], BF16)
                nc.vector.memset(zT.bitcast(F32), 0.0)
                nc.sync.dma_start(out=zT[36:37, 2, :], in_=onesr_d)
                nc.vector.memset(uT, 0.0)
                nc.sync.dma_start(out=uT[36:37, 2, :], in_=onesb_d[:, 0:64])
                chx = [(0, 128), (1, 128), (2, 36)]
                for c, w in chx:
                    nc.tensor.transpose(trp[0:w, c, :], zsb[:, c * 128:c * 128 + w], id64)
                    nc.scalar.copy(out=zT[0:w, c, :], in_=trp[0:w, c, :])
                xp = pp.tile([BC, D_LATENT], F32)
                for c in range(KCX):
                    nc.tensor.matmul(xp, zT[:, c, :], w1s[:, c, :],
                                     start=(c == 0), stop=(c == KCX - 1))
                # selu (scale folded into wih0): u = relu(x) + min(0, a*(exp(x)-1))
                esb = pt.tile([BC, D_LATENT], F32)
                nc.scalar.activation(esb, xp, ACTF.Exp)
                t1 = pt.tile([BC, D_LATENT], F32)
                nc.vector.tensor_scalar(
                    out=t1, in0=esb, scalar1=1.0, scalar2=SELU_A,
                    op0=ALU.subtract, op1=ALU.mult)
                t2 = pt.tile([BC, D_LATENT], F32)
                nc.vector.tensor_scalar(
                    out=t2, in0=t1, scalar1=0.0, scalar2=0.0,
                    op0=ALU.min, op1=ALU.bypass)
                usb = pt.tile([BC, D_LATENT], F32)
                nc.vector.scalar_tensor_tensor(
                    out=usb, in0=xp, scalar=0.0, in1=t2,
                    op0=ALU.max, op1=ALU.add)
                for c, w in chx:
                    nc.tensor.transpose(trp[0:w, c, :], usb[:, c * 128:c * 128 + w], id64)
                    nc.scalar.copy(out=uT[0:w, c, :], in_=trp[0:w, c, :])
                g0p = pp.tile([BC, 1536], F32)
                for c in range(KCX):
                    for lo, w in RZCH:
                        nc.tensor.matmul(g0p[:, lo:lo + w], uT[:, c, :],
                                         wih0[:, c, lo:lo + w],
                                         start=(c == 0), stop=(c == KCX - 1))
                    nc.tensor.matmul(g0p[:, 1024:1024 + N_W], uT[:, c, :],
                                     wih0[:, c, N_SRC:N_SRC + N_W],
                                     start=(c == 0), stop=(c == KCX - 1))
                nc.scalar.copy(out=gi0rz, in_=g0p[:, 0:1002])
                nc.scalar.copy(out=gi0n, in_=g0p[:, 1024:1024 + N_W])

            # ================= scan =================
            # PSUM map (words per partition, 4096 total):
            #   banks 0-3 (0:2048):  P12 [128, 2048]: c1=p0:64, c2=p64:128
            #       rz 0:1002 (gh+gi), n_gh 1024:1526, n_gi 1536:2038
            #   banks 4-6 (2048:3584): P0 [64, 1536]: rz 0:1002, n_gh 1024:1526
            #   bank 7 (3584:4096): ptr (bf16 transposes) + epilogue pf
            with tc.tile_pool(name="spsum", bufs=1, space="PSUM") as sp, \
                 tc.tile_pool(name="wk", bufs=2) as wk:
                P12 = sp.tile([128, 2048], F32)
                P0 = sp.tile([BC, 1536], F32)
                ptr = sp.tile([128, KC, 192], BF16)  # c12: 0:128, c0: 128:192

                chh = [(0, 128), (1, 128), (2, 128), (3, 117)]

                def cell_mms(pgh, hgT_prev, hgi_in, whh, wih):
                    """gh + gi matmuls for one cell into its psum region.
                    pgh: [*, 2048-view] (rz 0:1002, n_gh 1024, n_gi 1536)."""
                    for c in range(KC):
                        for lo, w in RZCH:
                            nc.tensor.matmul(pgh[:, lo:lo + w], hgT_prev[:, c, :],
                                             whh[:, c, lo:lo + w],
                                             start=(c == 0), stop=False)
                        nc.tensor.matmul(pgh[:, N_GH:N_GH + N_W], hgT_prev[:, c, :],
                                         whh[:, c, N_SRC:N_SRC + N_W],
                                         start=(c == 0), stop=(c == KC - 1))
                    for c in range(KC):
                        for lo, w in RZCH:
                            nc.tensor.matmul(pgh[:, lo:lo + w], hgi_in[:, c, :],
                                             wih[:, c, lo:lo + w],
                                             start=False, stop=(c == KC - 1))
                        nc.tensor.matmul(pgh[:, N_GI:N_GI + N_W], hgi_in[:, c, :],
                                         wih[:, c, N_SRC:N_SRC + N_W],
                                         start=(c == 0), stop=(c == KC - 1))

                def cell0_mms(pgh):
                    for c in range(KC):
                        for lo, w in RZCH:
                            nc.tensor.matmul(pgh[:, lo:lo + w], h0T[:, c, :],
                                             whh0[:, c, lo:lo + w],
                                             start=(c == 0), stop=False)
                        nc.tensor.matmul(pgh[:, N_GH:N_GH + N_W], h0T[:, c, :],
                                         whh0[:, c, N_SRC:N_SRC + N_W],
                                         start=(c == 0), stop=(c == KC - 1))
                    for lo, w in RZCH:  # gi0 via identity matmul (bf16 lhsT)
                        nc.tensor.matmul(pgh[:, lo:lo + w], identb[0:64, 0:64],
                                         gi0rz[:, lo:lo + w],
                                         start=False, stop=True)


                def ew_a(pgh, gin_psum, gin_sb, pw, sl):
                    """sigmoid(rz), u = r*gh_n, v = u + gi_n, n = tanh(v)."""
                    rz = wk.tile([128, 1002], BF16, tag=f"rz{pw}", name=f"rz{pw}")[sl]
                    nc.scalar.activation(rz, pgh[:, 0:1002], ACTF.Sigmoid)
                    u = wk.tile([128, H], BF16, tag=f"u{pw}", name=f"u{pw}")[sl]
                    nc.vector.scalar_tensor_tensor(
                        out=u, in0=pgh[:, N_GH:N_GH + H], scalar=0.0,
                        in1=rz[:, 0:H], op0=ALU.add, op1=ALU.mult)
                    v = wk.tile([128, H], BF16, tag=f"v{pw}", name=f"v{pw}")[sl]
                    if gin_psum is not None:
                        nc.vector.tensor_tensor(v, u, gin_psum, op=ALU.add)
                    else:
                        nc.vector.tensor_tensor(v, u, gin_sb[:, 0:H], op=ALU.add)
                    n = wk.tile([128, H], BF16, tag=f"n{pw}", name=f"n{pw}")[sl]
                    nc.scalar.activation(n, v, ACTF.Tanh)
                    return rz, n

                def ew_b(rz, n, hb, pw, sl):
                    """h' = n + z*(h - n), on Pool (own queue, DVE stays
                    free for the transpose copies)."""
                    d = wk.tile([128, H], BF16, tag=f"d{pw}", name=f"d{pw}")[sl]
                    nc.gpsimd.tensor_tensor(d, hb, n, op=ALU.subtract)
                    e = wk.tile([128, H], BF16, tag=f"e{pw}", name=f"e{pw}")[sl]
                    nc.gpsimd.tensor_tensor(e, rz[:, H:H + H], d, op=ALU.mult)
                    nc.gpsimd.tensor_tensor(hb, n, e, op=ALU.add)

                def trans(src, dst, po, idsl, hist_slot=None):
                    """PE-transpose src [64, H] into ptr cols [po:po+64],
                    then ONE fused DVE copy of all 4 chunks to dst."""
                    for c, w in chh:
                        nc.tensor.transpose(ptr[0:w, c, po:po + 64],
                                            src[:, c * 128:c * 128 + w],
                                            identb[idsl, idsl])
                    if hist_slot is not None:
                        nc.vector.tensor_copy(
                            hist2[:, hist_slot, 0:3, :], ptr[:, 0:3, po:po + 64])
                        nc.vector.tensor_copy(
                            hist2[0:117, hist_slot, 3, :], ptr[0:117, 3, po:po + 64])
                    else:
                        nc.vector.tensor_copy(dst[:, 0:3, :], ptr[:, 0:3, po:po + 64])
                        nc.vector.tensor_copy(dst[0:117, 3, :], ptr[0:117, 3, po:po + 64])

                # wave tau: cell0 ticks tau, cell1 tau-1, cell2 tau-3.
                # Emission order keeps every transpose-copy ahead of long EW
                # ops in the DVE queue and every EW finished one wave before
                # its transpose runs on PE.
                ew2 = ew0 = ew1 = None
                for tau in range(T + 4):
                    tk0, tk1, tk2 = tau, tau - 1, tau - 3
                    do0 = 0 <= tk0 < T
                    do1 = 0 <= tk1 < T
                    do2 = 0 <= tk2 < T
                    if do2:
                        cell_mms(P12[64:128, :], hist2[:, tk2, :, :],
                                 h1T, whh2, wih2)
                    if 0 <= tk0 - 1 < T:   # EW0(tau-1) output h0(tk0-1)
                        trans(h0b, h0T, 64, slice(0, 64))
                    if do2:
                        ew2 = ew_a(P12[64:128], P12[64:128, N_GI:N_GI + H],
                                   None, "2", slice(64, 128))
                    if do0:
                        cell0_mms(P0)
                    if 0 <= tk1 - 1 < T:   # EW1(tau-1) output h1(tk1-1)
                        trans(h12b[0:64], h1T, 128, slice(0, 64))
                    if do2:
                        ew_b(*ew2, h12b[64:128], "2", slice(64, 128))
                    if do0:
                        ew0 = ew_a(P0, None, gi0n, "0", slice(0, 64))
                    if do1:
                        cell_mms(P12[0:64, :], h1T, h0T, whh1, wih1)
                    if do2:                # EW2(tau) ready mid-wave
                        trans(h12b[64:128], None, 0, slice(64, 128),
                              hist_slot=tk2 + 1)
                    if do0:
                        ew_b(*ew0, h0b, "0", slice(0, 64))
                    if do1:
                        ew1 = ew_a(P12[0:64], P12[0:64, N_GI:N_GI + H],
                                   None, "1", slice(0, 64))
                        ew_b(*ew1, h12b[0:64], "1", slice(0, 64))

            # ================= epilogue: fc2 + softmax (from SBUF) ==========
            with tc.tile_pool(name="fpsum", bufs=2, space="PSUM") as fp, \
                 tc.tile_pool(name="fwk", bufs=3) as fw:
                nchunk = (T + EPI_TB - 1) // EPI_TB
                for g in range(nchunk):
                    t0 = g * EPI_TB
                    nt = min(EPI_TB, T - t0)
                    pf = fp.tile([BC, EPI_TB * 36], F32, tag="pf")
                    for i in range(nt):
                        for c in range(KC):
                            nc.tensor.matmul(
                                pf[:, i * 36:(i + 1) * 36],
                                hist2[:, 1 + t0 + i, c, :], w2s[:, c, :],
                                start=(c == 0), stop=(c == KC - 1))
                    e = fw.tile([BC, EPI_TB, 36], F32, tag="e")
                    nc.scalar.activation(
                        e.rearrange("p a b -> p (a b)"),
                        pf[:, 0:EPI_TB * 36], ACTF.Exp)
                    ssum = fw.tile([BC, EPI_TB, 1], F32, tag="ssum")
                    nc.vector.reduce_sum(
                        ssum, e[:, :, 0:D_CHAR], axis=AX.X)
                    rcp = fw.tile([BC, EPI_TB], F32, tag="rcp")
                    nc.vector.reciprocal(rcp, ssum.rearrange("p a b -> p (a b)"))
                    pb = fw.tile([BC, EPI_TB, D_CHAR], F32, tag="pb")
                    for i in range(nt):
                        nc.vector.tensor_scalar_mul(
                            pb[:, i, :], in0=e[:, i, 0:D_CHAR],
                            scalar1=rcp[:, i:i + 1])
                    nc.sync.dma_start(out=probs[:, t0:t0 + nt, :],
                                      in_=pb[:, 0:nt, :])

    nc.compile()
    return nc


def _prep_rec(w, b, kc, ones_row, dtype):
    """weight [Gout, Kin] + bias -> [128, kc, 1504] (bias on ones_row)."""
    gout, kin = w.shape
    gpad = gout + (gout % 2)
    arr = np.zeros((128, kc, gpad), dtype=np.float32)
    wt = np.ascontiguousarray(w.T)
    for c in range(kc):
        lo = c * 128
        hi = min(lo + 128, kin)
        if hi > lo:
            arr[0:hi - lo, c, 0:gout] = wt[lo:hi]
    c, p = divmod(ones_row, 128)
    arr[p, c, 0:gout] = b
    return arr.astype(dtype)


def make_in_maps(inputs):
    import ml_dtypes
    bf16 = ml_dtypes.bfloat16
    inputs = {k: np.asarray(v, dtype=np.float32) for k, v in inputs.items()}
    TS = T + 1
    shared = {
        "onesb": np.ones((1, TS * 64), dtype=bf16),
        "onesr": np.ones((1, 64), dtype=np.float32),
        "w1s": _prep_rec(inputs["fc1_w"], inputs["fc1_b"], KCX, ONES_ROW_X,
                         np.float32),
        "wih0s": _prep_rec(SELU_L * inputs["w_ih0"], inputs["b_ih0"], KCX,
                           ONES_ROW_X, bf16),
        "whh0s": _prep_rec(inputs["w_hh0"], inputs["b_hh0"], KC, ONES_ROW, bf16),
        "wih1s": _prep_rec(inputs["w_ih1"], inputs["b_ih1"], KC, ONES_ROW, bf16),
        "whh1s": _prep_rec(inputs["w_hh1"], inputs["b_hh1"], KC, ONES_ROW, bf16),
        "wih2s": _prep_rec(inputs["w_ih2"], inputs["b_ih2"], KC, ONES_ROW, bf16),
        "whh2s": _prep_rec(inputs["w_hh2"], inputs["b_hh2"], KC, ONES_ROW, bf16),
        "w2s": _prep_rec(inputs["fc2_w"], inputs["fc2_b"], KC, ONES_ROW,
                         bf16)[:, :, 0:36].copy(),
    }
    in_maps = []
    for i in range(NCORES):
        m = dict(shared)
        m["z_in"] = np.ascontiguousarray(inputs["z"][i * BC:(i + 1) * BC])
        in_maps.append(m)
    return in_maps


def kernel(**inputs):
    if "nc" not in _CACHE:
        _CACHE["nc"] = build_bass()
    nc = _CACHE["nc"]
    in_maps = make_in_maps(inputs)
    res = bass_utils.run_bass_kernel_spmd(nc, in_maps, list(range(NCORES)))
    out = np.concatenate([r["probs"] for r in res.results], axis=0)
    return out


if __name__ == "__main__":
    pass
